# revision 3
# baseline (speedup 1.0000x reference)
"""CrossEntropy + SNNL loss on 8 Trainium2 NeuronCores.

loss = CE(y_, y) + ALPHA * SNNL(x_r, y)

Strategy (self-contained; shapes hardcoded for B=8192, D=256, C=1000):
- Host: normalize x_r rows (fp32), sort the similarity-matrix columns by
  class label (the loss is permutation invariant), cast the normalized
  transposed matrix to bf16.
- Rows are dealt to cores in an interleaved pattern: block position b of
  core k holds sorted rows [1024b+128k, 1024b+128(k+1)), so every core's
  block b sees the same (global) set of present classes -> the reduce
  ranges baked into the shared SPMD program are valid on every core.
- Per 128-row block each core matmuls its [128, 8192] slab of
  sim = xn @ xn.T on the PE (bf16, fp32 PSUM, K=256 via two accumulating
  chunks, 2048-wide PSUM quarters ping-ponged). ScalarE computes
  E = exp(sim/Tp - 1/Tp) quarter-wise with accum_out giving the quarter
  row-sums (-> bot) for free. VectorE reduces only the <=3 class column
  ranges present at this block position (-> top), split at quarter
  boundaries and issued right after each quarter's exp. CE logit exps run
  at the tail of ScalarE's program (their DMAs are never urgent); their
  row sums are reduced on VectorE.
- Each core outputs a [128, 96] tile of per-row partial terms; the host
  (float64) finishes: bot/top assembly, logs, fallbacks, and the mean.
"""

import os

import numpy as np

T = 0.5
ALPHA = 0.1
EPS_T = 1e-6
EPS_N = 1e-8
B, D, C = 8192, 256, 1000
NCORES = 8
RPC = B // NCORES  # 1024 rows per core
NBLK = RPC // 128  # 8 row blocks per core
QW = 2048  # PSUM quarter width (4 banks of fp32)
NQ = B // QW  # 4 quarters per row block
NJ = 6  # top-reduce sub-job slots per block position

LAST_EXEC_NS = None


def _split_excess_waits(nc, limit=1):
    """Move sync waits this walrus build cannot encode onto same-engine NoOps.

    This walrus rejects any InstDrain carrying a sync wait, and instructions
    with more than one wait. Semantically identical: the engine blocks on the
    same semaphores immediately before the original instruction.
    """
    import concourse.mybir as mybir

    n_split = 0
    for f in nc.m.functions:
        for blk in f.blocks:
            il = blk.instructions
            i = 0
            while i < len(il):
                inst = il[i]
                si = getattr(inst, "sync_info", None)
                if si is None:
                    i += 1
                    continue
                is_drain = type(inst).__name__ == "InstDrain"
                lim = 0 if is_drain else limit
                if len(si.on_wait) > lim:
                    waits = list(si.on_wait)
                    keep = waits[len(waits) - lim :] if lim else []
                    movew = waits[: len(waits) - lim]
                    inst.sync_info = mybir.SyncInfo(
                        on_wait=keep, on_update=list(si.on_update)
                    )
                    for j in range(0, len(movew), max(limit, 1)):
                        nd = mybir.InstNoOp(name=f"wsplit-{n_split}")
                        n_split += 1
                        nd.engine = inst.engine
                        nd.sync_info = mybir.SyncInfo(
                            on_wait=movew[j : j + max(limit, 1)], on_update=[]
                        )
                        il.insert(i, nd)
                        i += 1
                i += 1
    return n_split


def _build_bass(subjobs):
    """Build the single SPMD Bass program shared by all 8 cores.

    subjobs[b][q]: list of (lo, hi, slot) global column ranges inside
    quarter q whose sums VectorE computes for block position b; identical
    on every core by construction of the row deal.
    """
    import concourse.bass as bass
    import concourse.tile as tile
    from concourse import mybir

    F32 = mybir.dt.float32
    BF16 = mybir.dt.bfloat16
    AF = mybir.ActivationFunctionType
    AX = mybir.AxisListType

    Tp = T + EPS_T
    scale = 1.0 / Tp

    nc = bass.Bass(enable_partition_id=False)
    xnt = nc.dram_tensor("xnt", [2, 128, B], BF16, kind="ExternalInput")
    lhst = nc.dram_tensor("lhst", [2, 128, RPC], BF16, kind="ExternalInput")
    ylog = nc.dram_tensor("ylog", [NBLK, 128, C], BF16, kind="ExternalInput")
    terms = nc.dram_tensor("terms", [128, 96], F32, kind="ExternalOutput")

    with tile.TileContext(nc) as tc:
        with (
            tc.tile_pool(name="const", bufs=1) as const,
            tc.tile_pool(name="epool", bufs=3) as epool,
            tc.tile_pool(name="cpool", bufs=2) as cpool,
            tc.tile_pool(name="psum", bufs=2, space="PSUM") as psum,
        ):
            xnt_t = const.tile([128, 2, B], BF16)
            lhst_t = const.tile([128, 2, RPC], BF16)
            ylog_t = const.tile([128, NBLK, C], BF16)
            ebias = const.tile([128, 1], F32)
            dummy = const.tile([128, 1], F32)
            # out tile: 0:32 bot quarter accums (4/blk), 32:80 top sub-jobs
            # (6 slots/blk), 80:88 CE sumexp (1/blk), 88:91 block-0 first
            # quarter extra pieces
            outt = const.tile([128, 96], F32)

            # DMA order == queue priority. Front: block-0 weights (tiny),
            # first xnt quarter in small chunks (PE gate), then the rest of
            # the working set; logits last (their consumers run last).
            for kc in range(2):
                nc.sync.dma_start(lhst_t[:, kc, 0:128], lhst[kc, :, 0:128])
            for h in range(4):
                for kc in range(2):
                    nc.sync.dma_start(
                        xnt_t[:, kc, 512 * h : 512 * (h + 1)],
                        xnt[kc, :, 512 * h : 512 * (h + 1)],
                    )
            for kc in range(2):
                nc.sync.dma_start(lhst_t[:, kc, 128:RPC], lhst[kc, :, 128:RPC])
            for q in range(1, NQ):
                for kc in range(2):
                    nc.sync.dma_start(
                        xnt_t[:, kc, QW * q : QW * (q + 1)],
                        xnt[kc, :, QW * q : QW * (q + 1)],
                    )
            for b in range(NBLK):
                nc.sync.dma_start(ylog_t[:, b, :], ylog[b, :, :])
            nc.vector.memset(ebias, -scale)
            nc.vector.memset(outt[:, 32:80], 0.0)
            # preload the exp ACT table off the critical path
            nc.scalar.activation(out=dummy, in_=ebias, func=AF.Exp)

            for b in range(NBLK):
                # rotate quarter order so the quarters holding this block's
                # top ranges are processed first (kills the DVE tail)
                qorder = [(b // 2 + dq) % NQ for dq in range(NQ)]
                eb = epool.tile([128, B], BF16, tag="eb")
                for q in qorder:
                    pq = psum.tile([128, QW], F32, tag="pq")
                    for kc in range(2):
                        lw = lhst_t[:, kc, 128 * b : 128 * (b + 1)]
                        for t in range(QW // 512):
                            nc.tensor.matmul(
                                pq[:, 512 * t : 512 * (t + 1)],
                                lw,
                                xnt_t[:, kc, QW * q + 512 * t : QW * q + 512 * (t + 1)],
                                start=(kc == 0),
                                stop=(kc == 1),
                            )
                    if b == 0 and q == qorder[0]:
                        # smaller pieces track the PE's cold-start ramp
                        for t in range(4):
                            acol = (4 * b + q) if t == 0 else (87 + t)
                            nc.scalar.activation(
                                out=eb[:, QW * q + 512 * t : QW * q + 512 * (t + 1)],
                                in_=pq[:, 512 * t : 512 * (t + 1)],
                                func=AF.Exp,
                                bias=ebias,
                                scale=scale,
                                accum_out=outt[:, acol : acol + 1],
                            )
                    else:
                        nc.scalar.activation(
                            out=eb[:, QW * q : QW * (q + 1)],
                            in_=pq,
                            func=AF.Exp,
                            bias=ebias,
                            scale=scale,
                            accum_out=outt[:, 4 * b + q : 4 * b + q + 1],
                        )
                    # top sub-jobs inside this quarter -> their slots
                    for lo, hi, slot in subjobs[b][q]:
                        nc.vector.reduce_sum(
                            out=outt[:, 32 + NJ * b + slot : 33 + NJ * b + slot],
                            in_=eb[:, lo:hi],
                            axis=AX.X,
                        )

            # ---- CE: max-free logsumexp; exps at the ScalarE tail ----
            for b in range(NBLK):
                esc = cpool.tile([128, C], BF16, tag="esc")
                nc.scalar.activation(
                    out=esc, in_=ylog_t[:, b, :], func=AF.Exp, bias=0.0, scale=1.0
                )
                nc.vector.reduce_sum(
                    out=outt[:, 80 + b : 81 + b], in_=esc, axis=AX.X
                )

            nc.sync.dma_start(terms[:, :], outt)

    return nc


def kernel(x_r, y_, y):
    global LAST_EXEC_NS
    import ml_dtypes
    from concourse.bass_utils import run_bass_kernel_spmd

    x_r = np.asarray(x_r, dtype=np.float32)
    y_ = np.asarray(y_, dtype=np.float32)
    y = np.asarray(y).astype(np.int64)

    # ---- host prep: normalize, sort columns by class ----
    norms = np.maximum(np.linalg.norm(x_r, axis=1, keepdims=True), EPS_N).astype(
        np.float32
    )
    xn = (x_r / norms).astype(np.float32)
    perm = np.argsort(y, kind="stable")
    y_perm = y[perm]
    classes, counts = np.unique(y_perm, return_counts=True)
    offs = np.concatenate([[0], np.cumsum(counts)])
    cls_lo = {int(c): int(offs[i]) for i, c in enumerate(classes)}
    cls_hi = {int(c): int(offs[i + 1]) for i, c in enumerate(classes)}
    cls_cnt = {int(c): int(n) for c, n in zip(classes, counts)}

    # per block position b: classes present in sorted window
    # [1024b, 1024b+1024); their column ranges split at quarter boundaries
    subjobs = []  # [b][q] -> list of (lo, hi, slot)
    slot_of = []  # [b] -> {class: [slots]}
    for b in range(NBLK):
        wcls = sorted(set(int(c) for c in y_perm[1024 * b : 1024 * (b + 1)]))
        sj = [[] for _ in range(NQ)]
        smap = {}
        slot = 0
        for c in wcls:
            smap[c] = []
            lo, hi = cls_lo[c], cls_hi[c]
            for q in range(lo // QW, (hi - 1) // QW + 1):
                qlo, qhi = max(lo, QW * q), min(hi, QW * (q + 1))
                sj[q].append((qlo, qhi, slot))
                smap[c].append(slot)
                slot += 1
        assert slot <= NJ, f"block {b} needs {slot} sub-jobs > NJ={NJ}"
        subjobs.append(sj)
        slot_of.append(smap)

    xnT = np.ascontiguousarray(xn[perm].T).astype(ml_dtypes.bfloat16)  # [256, 8192]
    xnt_in = np.ascontiguousarray(xnT.reshape(2, 128, B))

    in_maps = []
    core_rows = []  # per core: [NBLK, 128] global original row indices
    for k in range(NCORES):
        rows = np.stack(
            [perm[1024 * b + 128 * k : 1024 * b + 128 * (k + 1)] for b in range(NBLK)]
        )  # [NBLK, 128]
        core_rows.append(rows)
        lhst_in = np.ascontiguousarray(
            np.concatenate(
                [
                    xnt_in[:, :, 1024 * b + 128 * k : 1024 * b + 128 * (k + 1)]
                    for b in range(NBLK)
                ],
                axis=2,
            )
        )
        ylog_in = np.ascontiguousarray(y_[rows].astype(ml_dtypes.bfloat16))
        in_maps.append({"xnt": xnt_in, "lhst": lhst_in, "ylog": ylog_in})

    nc = _build_bass(subjobs)
    _split_excess_waits(nc)

    trace = bool(os.environ.get("SNNL_TRACE"))
    try:
        res = run_bass_kernel_spmd(
            nc, in_maps, core_ids=list(range(NCORES)), trace=trace
        )
    except Exception:
        # transient NRT/device failures (e.g. NRT_EXEC_UNIT_UNRECOVERABLE)
        # have been observed to succeed on retry
        import time

        time.sleep(2.0)
        res = run_bass_kernel_spmd(
            nc, in_maps, core_ids=list(range(NCORES)), trace=trace
        )
    LAST_EXEC_NS = res.exec_time_ns

    # ---- host finish (float64): assemble bot/top, logs, means ----
    ce_sum = 0.0
    sn_sum = 0.0
    for k, r in enumerate(res.results):
        t = np.asarray(r["terms"], dtype=np.float64)
        rows = core_rows[k]
        for b in range(NBLK):
            rb = rows[b]
            cls = y[rb]
            bot = t[:, 4 * b : 4 * b + 4].sum(axis=1) - 1.0
            if b == 0:
                bot = bot + t[:, 88:91].sum(axis=1)
            tj = t[:, 32 + NJ * b : 32 + NJ * (b + 1)]
            top = np.empty(128)
            for i in range(128):
                top[i] = sum(tj[i, s] for s in slot_of[b][int(cls[i])])
            top -= 1.0
            has_pos = np.array([cls_cnt[int(c)] - 1 > 0 for c in cls])
            top = np.where(has_pos, top, 1e-6)
            sn_sum += np.sum(np.log(top / bot))
            sumexp = t[:, 80 + b]
            ysel = y_[rb, y[rb]].astype(np.float64)
            ce_sum += np.sum(np.log(sumexp) - ysel)
    loss = ce_sum / B - ALPHA * (sn_sum / B)
    return np.array(loss, dtype=np.float32)


# revision 5
# speedup vs baseline: 1.3729x; 1.3729x over previous
"""CrossEntropy + SNNL loss on 8 Trainium2 NeuronCores.

loss = CE(y_, y) + ALPHA * SNNL(x_r, y)

Strategy (self-contained; shapes hardcoded for B=8192, D=256, C=1000):
- Host: normalize x_r rows (fp32), sort the similarity-matrix columns by
  class label (the loss is permutation invariant), cast the normalized
  transposed matrix to bf16.
- Rows are dealt to cores in an interleaved pattern: block position b of
  core k holds sorted rows [1024b+128k, 1024b+128(k+1)), so every core's
  block b sees the same (global) set of present classes -> the reduce
  ranges baked into the shared SPMD program are valid on every core.
- Per 128-row block each core matmuls its [128, 8192] slab of
  sim = xn @ xn.T on the PE (bf16, fp32 PSUM, K=256 via two accumulating
  chunks, 2048-wide PSUM quarters ping-ponged). ScalarE computes
  E = exp(sim/Tp - 1/Tp) quarter-wise with accum_out giving the quarter
  row-sums (-> bot) for free. VectorE reduces only the <=3 class column
  ranges present at this block position (-> top), split at quarter
  boundaries and issued right after each quarter's exp. CE logit exps run
  at the tail of ScalarE's program (their DMAs are never urgent); their
  row sums are reduced on VectorE.
- Each core outputs a [128, 96] tile of per-row partial terms; the host
  (float64) finishes: bot/top assembly, logs, fallbacks, and the mean.
"""

import os

import numpy as np

T = 0.5
ALPHA = 0.1
EPS_T = 1e-6
EPS_N = 1e-8
B, D, C = 8192, 256, 1000
NCORES = 8
RPC = B // NCORES  # 1024 rows per core
NBLK = RPC // 128  # 8 row blocks per core
QW = 2048  # PSUM quarter width (4 banks of fp32)
NQ = B // QW  # 4 quarters per row block
NJ = 6  # top-reduce sub-job slots per block position

LAST_EXEC_NS = None


def _split_excess_waits(nc, limit=1):
    """Move sync waits this walrus build cannot encode onto same-engine NoOps.

    This walrus rejects any InstDrain carrying a sync wait, and instructions
    with more than one wait. Semantically identical: the engine blocks on the
    same semaphores immediately before the original instruction.
    """
    import concourse.mybir as mybir

    n_split = 0
    for f in nc.m.functions:
        for blk in f.blocks:
            il = blk.instructions
            i = 0
            while i < len(il):
                inst = il[i]
                si = getattr(inst, "sync_info", None)
                if si is None:
                    i += 1
                    continue
                is_drain = type(inst).__name__ == "InstDrain"
                lim = 0 if is_drain else limit
                if len(si.on_wait) > lim:
                    waits = list(si.on_wait)
                    keep = waits[len(waits) - lim :] if lim else []
                    movew = waits[: len(waits) - lim]
                    inst.sync_info = mybir.SyncInfo(
                        on_wait=keep, on_update=list(si.on_update)
                    )
                    for j in range(0, len(movew), max(limit, 1)):
                        nd = mybir.InstNoOp(name=f"wsplit-{n_split}")
                        n_split += 1
                        nd.engine = inst.engine
                        nd.sync_info = mybir.SyncInfo(
                            on_wait=movew[j : j + max(limit, 1)], on_update=[]
                        )
                        il.insert(i, nd)
                        i += 1
                i += 1
    return n_split


def _build_bass(subjobs):
    """Build the single SPMD Bass program shared by all 8 cores.

    subjobs[b][q]: list of (lo, hi, slot) global column ranges inside
    quarter q whose sums VectorE computes for block position b; identical
    on every core by construction of the row deal.
    """
    import concourse.bass as bass
    import concourse.tile as tile
    from concourse import mybir

    F32 = mybir.dt.float32
    BF16 = mybir.dt.bfloat16
    AF = mybir.ActivationFunctionType
    AX = mybir.AxisListType

    Tp = T + EPS_T
    scale = 1.0 / Tp

    nc = bass.Bass(enable_partition_id=False)
    xnt = nc.dram_tensor("xnt", [2, 128, B], BF16, kind="ExternalInput")
    lhst = nc.dram_tensor("lhst", [2, 128, RPC], BF16, kind="ExternalInput")
    ylog = nc.dram_tensor("ylog", [NBLK, 128, C], BF16, kind="ExternalInput")
    terms = nc.dram_tensor("terms", [128, 96], F32, kind="ExternalOutput")

    with tile.TileContext(nc) as tc:
        with (
            tc.tile_pool(name="const", bufs=1) as const,
            tc.tile_pool(name="epool", bufs=3) as epool,
            tc.tile_pool(name="cpool", bufs=2) as cpool,
            tc.tile_pool(name="psum", bufs=2, space="PSUM") as psum,
        ):
            xnt_t = const.tile([128, 2, B], BF16)
            lhst_t = const.tile([128, 2, RPC], BF16)
            ylog_t = const.tile([128, NBLK, C], BF16)
            ebias = const.tile([128, 1], F32)
            dummy = const.tile([128, 1], F32)
            # out tile: 0:32 bot quarter accums (4/blk), 32:80 top sub-jobs
            # (6 slots/blk), 80:88 CE sumexp (1/blk), 88:91 block-0 first
            # quarter extra pieces
            outt = const.tile([128, 96], F32)

            # DMA order == queue priority. Front: block-0 weights (tiny) and
            # the first rhs chunk in 32KB pieces (PE gate), then the rest of
            # xnt at one 128KB chunk per matmul so each MM is gated only on
            # its own chunk; logits last (their consumers run last).
            for kc in range(2):
                nc.sync.dma_start(lhst_t[:, kc, 0:128], lhst[kc, :, 0:128])
            for kc in range(2):
                for h in range(4):
                    nc.sync.dma_start(
                        xnt_t[:, kc, 128 * h : 128 * (h + 1)],
                        xnt[kc, :, 128 * h : 128 * (h + 1)],
                    )
            for h in range(1, 4):
                for kc in range(2):
                    nc.sync.dma_start(
                        xnt_t[:, kc, 512 * h : 512 * (h + 1)],
                        xnt[kc, :, 512 * h : 512 * (h + 1)],
                    )
            for kc in range(2):
                nc.sync.dma_start(lhst_t[:, kc, 128:RPC], lhst[kc, :, 128:RPC])
            for q in range(1, NQ):
                for t in range(4):
                    for kc in range(2):
                        nc.sync.dma_start(
                            xnt_t[:, kc, QW * q + 512 * t : QW * q + 512 * (t + 1)],
                            xnt[kc, :, QW * q + 512 * t : QW * q + 512 * (t + 1)],
                        )
            for b in range(NBLK):
                nc.sync.dma_start(ylog_t[:, b, :], ylog[b, :, :])
            nc.vector.memset(ebias, -scale)
            nc.vector.memset(outt[:, 32:80], 0.0)
            # preload the exp ACT table off the critical path
            nc.scalar.activation(out=dummy, in_=ebias, func=AF.Exp)

            for b in range(NBLK):
                # rotate quarter order so the quarters holding this block's
                # top ranges are processed first (kills the DVE tail)
                qorder = [(b // 2 + dq) % NQ for dq in range(NQ)]
                eb = epool.tile([128, B], BF16, tag="eb")
                for q in qorder:
                    pq = psum.tile([128, QW], F32, tag="pq")
                    for t in range(QW // 512):
                        for kc in range(2):
                            nc.tensor.matmul(
                                pq[:, 512 * t : 512 * (t + 1)],
                                lhst_t[:, kc, 128 * b : 128 * (b + 1)],
                                xnt_t[:, kc, QW * q + 512 * t : QW * q + 512 * (t + 1)],
                                start=(kc == 0),
                                stop=(kc == 1),
                            )
                    if b == 0 and q == qorder[0]:
                        # smaller pieces track the PE's cold-start ramp
                        for t in range(4):
                            acol = (4 * b + q) if t == 0 else (87 + t)
                            nc.scalar.activation(
                                out=eb[:, QW * q + 512 * t : QW * q + 512 * (t + 1)],
                                in_=pq[:, 512 * t : 512 * (t + 1)],
                                func=AF.Exp,
                                bias=ebias,
                                scale=scale,
                                accum_out=outt[:, acol : acol + 1],
                            )
                    else:
                        nc.scalar.activation(
                            out=eb[:, QW * q : QW * (q + 1)],
                            in_=pq,
                            func=AF.Exp,
                            bias=ebias,
                            scale=scale,
                            accum_out=outt[:, 4 * b + q : 4 * b + q + 1],
                        )
                    # top sub-jobs inside this quarter -> their slots
                    for lo, hi, slot in subjobs[b][q]:
                        nc.vector.reduce_sum(
                            out=outt[:, 32 + NJ * b + slot : 33 + NJ * b + slot],
                            in_=eb[:, lo:hi],
                            axis=AX.X,
                        )

            # ---- CE: max-free logsumexp; exps at the ScalarE tail ----
            for b in range(NBLK):
                esc = cpool.tile([128, C], BF16, tag="esc")
                nc.scalar.activation(
                    out=esc, in_=ylog_t[:, b, :], func=AF.Exp, bias=0.0, scale=1.0
                )
                nc.vector.reduce_sum(
                    out=outt[:, 80 + b : 81 + b], in_=esc, axis=AX.X
                )

            nc.sync.dma_start(terms[:, :], outt)

    return nc


def kernel(x_r, y_, y):
    global LAST_EXEC_NS
    import ml_dtypes
    from concourse.bass_utils import run_bass_kernel_spmd

    x_r = np.asarray(x_r, dtype=np.float32)
    y_ = np.asarray(y_, dtype=np.float32)
    y = np.asarray(y).astype(np.int64)

    # ---- host prep: normalize, sort columns by class ----
    norms = np.maximum(np.linalg.norm(x_r, axis=1, keepdims=True), EPS_N).astype(
        np.float32
    )
    xn = (x_r / norms).astype(np.float32)
    perm = np.argsort(y, kind="stable")
    y_perm = y[perm]
    classes, counts = np.unique(y_perm, return_counts=True)
    offs = np.concatenate([[0], np.cumsum(counts)])
    cls_lo = {int(c): int(offs[i]) for i, c in enumerate(classes)}
    cls_hi = {int(c): int(offs[i + 1]) for i, c in enumerate(classes)}
    cls_cnt = {int(c): int(n) for c, n in zip(classes, counts)}

    # per block position b: classes present in sorted window
    # [1024b, 1024b+1024); their column ranges split at quarter boundaries
    subjobs = []  # [b][q] -> list of (lo, hi, slot)
    slot_of = []  # [b] -> {class: [slots]}
    for b in range(NBLK):
        wcls = sorted(set(int(c) for c in y_perm[1024 * b : 1024 * (b + 1)]))
        sj = [[] for _ in range(NQ)]
        smap = {}
        slot = 0
        for c in wcls:
            smap[c] = []
            lo, hi = cls_lo[c], cls_hi[c]
            for q in range(lo // QW, (hi - 1) // QW + 1):
                qlo, qhi = max(lo, QW * q), min(hi, QW * (q + 1))
                sj[q].append((qlo, qhi, slot))
                smap[c].append(slot)
                slot += 1
        assert slot <= NJ, f"block {b} needs {slot} sub-jobs > NJ={NJ}"
        subjobs.append(sj)
        slot_of.append(smap)

    xnT = np.ascontiguousarray(xn[perm].T).astype(ml_dtypes.bfloat16)  # [256, 8192]
    xnt_in = np.ascontiguousarray(xnT.reshape(2, 128, B))

    in_maps = []
    core_rows = []  # per core: [NBLK, 128] global original row indices
    for k in range(NCORES):
        rows = np.stack(
            [perm[1024 * b + 128 * k : 1024 * b + 128 * (k + 1)] for b in range(NBLK)]
        )  # [NBLK, 128]
        core_rows.append(rows)
        lhst_in = np.ascontiguousarray(
            np.concatenate(
                [
                    xnt_in[:, :, 1024 * b + 128 * k : 1024 * b + 128 * (k + 1)]
                    for b in range(NBLK)
                ],
                axis=2,
            )
        )
        ylog_in = np.ascontiguousarray(y_[rows].astype(ml_dtypes.bfloat16))
        in_maps.append({"xnt": xnt_in, "lhst": lhst_in, "ylog": ylog_in})

    nc = _build_bass(subjobs)
    _split_excess_waits(nc)

    trace = bool(os.environ.get("SNNL_TRACE"))
    try:
        res = run_bass_kernel_spmd(
            nc, in_maps, core_ids=list(range(NCORES)), trace=trace
        )
    except Exception:
        # transient NRT/device failures (e.g. NRT_EXEC_UNIT_UNRECOVERABLE)
        # have been observed to succeed on retry
        import time

        time.sleep(2.0)
        res = run_bass_kernel_spmd(
            nc, in_maps, core_ids=list(range(NCORES)), trace=trace
        )
    LAST_EXEC_NS = res.exec_time_ns

    # ---- host finish (float64): assemble bot/top, logs, means ----
    ce_sum = 0.0
    sn_sum = 0.0
    for k, r in enumerate(res.results):
        t = np.asarray(r["terms"], dtype=np.float64)
        rows = core_rows[k]
        for b in range(NBLK):
            rb = rows[b]
            cls = y[rb]
            bot = t[:, 4 * b : 4 * b + 4].sum(axis=1) - 1.0
            if b == 0:
                bot = bot + t[:, 88:91].sum(axis=1)
            tj = t[:, 32 + NJ * b : 32 + NJ * (b + 1)]
            top = np.empty(128)
            for i in range(128):
                top[i] = sum(tj[i, s] for s in slot_of[b][int(cls[i])])
            top -= 1.0
            has_pos = np.array([cls_cnt[int(c)] - 1 > 0 for c in cls])
            top = np.where(has_pos, top, 1e-6)
            sn_sum += np.sum(np.log(top / bot))
            sumexp = t[:, 80 + b]
            ysel = y_[rb, y[rb]].astype(np.float64)
            ce_sum += np.sum(np.log(sumexp) - ysel)
    loss = ce_sum / B - ALPHA * (sn_sum / B)
    return np.array(loss, dtype=np.float32)
